# revision 1
# baseline (speedup 1.0000x reference)
"""GCN layer (SpMM + linear) on 8 Trainium2 NeuronCores — exact-packed dest-banded tiles, host-dense selection (bf16).

out[i] = (sum_{e: edge_row[e]==i} edge_val[e] * x[edge_col[e]]) @ W.T + b

Destination rows are partitioned across 8 cores (6250 each) into 13 PSUM
groups of 496 rows.  Per (group, source-half) bucket, edges are sorted by
destination and packed 128 per gather tile — no per-window padding.  Tile t's
destinations fall in a narrow data-derived band [db[t], db[t]+BW); the
selection matrix sval[slot, dest-db] (bf16, host-precomputed, resident in
SBUF) is dense over the band, so duplicate (src,dst) edges just sum.

Each slot gathers one 256B bf16 x row via SWDGE dma_gather (int16 indices,
lo/hi source halves on separate queues).  matmul(lhsT=gathered, rhs=sval
band) accumulates agg.T[feat, dest] into the group's PSUM bank.  Epilogue per
group: copy to SBUF, project with W.T (fp32), add bias, DMA out.
"""

import math
from dataclasses import dataclass

import numpy as np

GR = 496          # dest rows per PSUM group
CAP = 128         # slots per gather tile
D = 128           # feature dim


@dataclass(frozen=True)
class Cfg:
    n_nodes: int = 50000
    n_edges: int = 800000
    n_cores: int = 8
    chunk: int = 8            # tiles per dma_gather call
    gather_queues: int = 4
    gather_bufs: int = 8
    agg_bufs: int = 2         # PSUM agg groups in flight
    gelem: int = 128          # gather elems per descriptor (diagnostic only)
    dma_scratch: int = 16384  # SWDGE descriptor ring carveout bytes/partition
    skip_compute: bool = False
    loop_n: int = 0
    repeats: int = 1

    @property
    def split(self) -> int:
        return self.n_nodes // 2

    @property
    def rows_per_core(self) -> int:
        return self.n_nodes // self.n_cores

    @property
    def n_groups(self) -> int:
        return math.ceil(self.rows_per_core / GR)


@dataclass(frozen=True)
class Plan:
    tc: tuple    # tc[g][s]: tiles per (group, stream), max over cores
    db: tuple    # db[g][s][t]: band start column of tile t
    bw: int      # band width (compiled free size of the sval matmul)

    def t_stream(self, s):
        return sum(t[s] for t in self.tc)


def _preprocess(cfg: Cfg, x, edge_row, edge_col, edge_val, W, b):
    import ml_dtypes

    RPC = cfg.rows_per_core
    NG = cfg.n_groups
    SPLIT = cfg.split

    x = np.asarray(x)
    edge_row = np.asarray(edge_row)
    edge_col = np.asarray(edge_col)
    edge_val = np.asarray(edge_val)

    xb = np.ascontiguousarray(x.astype(ml_dtypes.bfloat16))
    xplo = np.ascontiguousarray(xb[:SPLIT])
    xphi = np.ascontiguousarray(xb[SPLIT:])

    # Pass 1: per (core, group, stream) dest-sorted edge arrays.
    per_core = []
    for c in range(cfg.n_cores):
        e0, e1 = np.searchsorted(edge_row, [c * RPC, (c + 1) * RPC])
        r_loc = edge_row[e0:e1] - c * RPC
        g_of_e = r_loc // GR
        d_loc = r_loc - g_of_e * GR
        src = edge_col[e0:e1].astype(np.int64)
        s_of_e = (src >= SPLIT).astype(np.int64)
        val = edge_val[e0:e1].astype(np.float64)
        buckets = {}
        for g in range(NG):
            for s in range(2):
                m = (g_of_e == g) & (s_of_e == s)
                order = np.argsort(d_loc[m], kind="stable")
                buckets[(g, s)] = (src[m][order] - s * SPLIT,
                                  d_loc[m][order], val[m][order])
        per_core.append(buckets)

    # Plan: tile counts, band starts, band width (shared across cores).
    tc = []
    for g in range(NG):
        tc.append(tuple(
            max(math.ceil(len(pc[(g, s)][0]) / CAP) for pc in per_core)
            for s in range(2)))

    db = []
    bw = 0
    for g in range(NG):
        row = []
        for s in range(2):
            nt = tc[g][s]
            starts = np.full(nt, GR, dtype=np.int64)
            ends = np.zeros(nt, dtype=np.int64)
            for pc in per_core:
                dd = pc[(g, s)][1]
                for t in range(nt):
                    seg = dd[t * CAP:(t + 1) * CAP]
                    if len(seg):
                        starts[t] = min(starts[t], seg[0])
                        ends[t] = max(ends[t], seg[-1] + 1)
            starts = np.minimum(starts, ends)  # empty tiles -> band at end
            bw = max(bw, int((ends - starts).max(initial=0)))
            row.append(tuple(int(v) for v in starts))
        db.append(tuple(row))
    bw = math.ceil(bw / 16) * 16
    # clamp band starts so db+bw stays inside the group
    db = tuple(
        tuple(tuple(min(v, GR - bw) for v in row_s) for row_s in row)
        for row in db)
    plan = Plan(tc=tuple(tc), db=db, bw=bw)

    CH = cfg.chunk
    tp = [math.ceil(plan.t_stream(s) / CH) * CH for s in range(2)]

    wt = np.asarray(W).T.astype(np.float32)
    bb = np.tile(np.asarray(b)[None, :].astype(np.float32), (128, 1))

    def wrap_idx(idx_lin):
        return np.tile(np.ascontiguousarray(idx_lin.reshape(-1, 16).T), (8, 1))

    in_maps = []
    for c in range(cfg.n_cores):
        buckets = per_core[c]
        idx_lin = [np.zeros(tp[s] * CAP, dtype=np.int16) for s in range(2)]
        sval = [np.zeros((128, tp[s] * bw), dtype=np.float64) for s in range(2)]
        spos = [0, 0]
        for g in range(NG):
            for t in range(max(plan.tc[g])):
                for s in range(2):
                    if t >= plan.tc[g][s]:
                        continue
                    ss, dd, vv = buckets[(g, s)]
                    ss = ss[t * CAP:(t + 1) * CAP]
                    dd = dd[t * CAP:(t + 1) * CAP]
                    vv = vv[t * CAP:(t + 1) * CAP]
                    pos = spos[s]
                    base = plan.db[g][s][t]
                    idx_lin[s][pos * CAP:pos * CAP + len(ss)] = ss
                    np.add.at(sval[s],
                              (np.arange(len(ss)), pos * bw + dd - base), vv)
                    spos[s] += 1
        in_maps.append({
            "xplo": xplo, "xphi": xphi,
            "gilo": np.ascontiguousarray(wrap_idx(idx_lin[0])),
            "gihi": np.ascontiguousarray(wrap_idx(idx_lin[1])),
            "svlo": np.ascontiguousarray(
                sval[0].astype(np.float32).astype(
                    np.asarray(xb).dtype)),
            "svhi": np.ascontiguousarray(
                sval[1].astype(np.float32).astype(
                    np.asarray(xb).dtype)),
            "cst": np.ascontiguousarray(
                np.concatenate([bb, wt], axis=1), dtype=np.float32),
        })
    return plan, in_maps


def _build_nc(cfg: Cfg, plan: Plan):
    from contextlib import ExitStack

    import concourse.bacc as bacc
    import concourse.mybir as mybir
    import concourse.tile as tile

    f32 = mybir.dt.float32
    bf16 = mybir.dt.bfloat16
    i16 = mybir.dt.int16
    NG = cfg.n_groups
    RPC = cfg.rows_per_core
    CH = cfg.chunk
    SPLIT = cfg.split
    BW = plan.bw
    tp = [math.ceil(plan.t_stream(s) / CH) * CH for s in range(2)]

    CW = 2 * D
    O_BB, O_WT = 0, D

    nc = bacc.Bacc("TRN2", target_bir_lowering=False,
                   num_swdge_queues=cfg.gather_queues,
                   dynamic_dma_scratch_size=cfg.dma_scratch)
    xp = [nc.dram_tensor("xplo", [SPLIT, D], bf16, kind="ExternalInput"),
          nc.dram_tensor("xphi", [SPLIT, D], bf16, kind="ExternalInput")]
    gi = [nc.dram_tensor("gilo", [128, tp[0] * 8], i16, kind="ExternalInput"),
          nc.dram_tensor("gihi", [128, tp[1] * 8], i16, kind="ExternalInput")]
    sv = [nc.dram_tensor("svlo", [128, tp[0] * BW], bf16,
                         kind="ExternalInput"),
          nc.dram_tensor("svhi", [128, tp[1] * BW], bf16,
                         kind="ExternalInput")]
    cst = nc.dram_tensor("cst", [128, CW], f32, kind="ExternalInput")
    y = nc.dram_tensor("y", [NG * GR, D], f32, kind="ExternalOutput")

    with tile.TileContext(nc) as tc, ExitStack() as ctx:
        const = ctx.enter_context(tc.tile_pool(name="const", bufs=1))
        gpool = [ctx.enter_context(tc.tile_pool(name="glo",
                                                bufs=cfg.gather_bufs)),
                 ctx.enter_context(tc.tile_pool(name="ghi",
                                                bufs=cfg.gather_bufs))]
        epool = ctx.enter_context(tc.tile_pool(name="epilog", bufs=2))
        ps_agg = ctx.enter_context(tc.tile_pool(name="psagg", bufs=cfg.agg_bufs,
                                                space="PSUM"))
        ps_out = ctx.enter_context(tc.tile_pool(name="psout", bufs=2,
                                                space="PSUM"))

        cst_sb = const.tile([128, CW], f32)
        nc.sync.dma_start(out=cst_sb[:], in_=cst[:])

        def bb_ap(p):
            return cst_sb[:p, O_BB:O_BB + D]

        def wt_ap():
            return cst_sb[:, O_WT:O_WT + D]

        gi_sb = []
        sv_sb = []
        for s in range(2):
            t = const.tile([128, tp[s] * 8], i16, tag=f"gi{s}")
            nc.sync.dma_start(out=t[:], in_=gi[s][:])
            gi_sb.append(t)
            t = const.tile([128, tp[s] * BW], bf16, tag=f"sv{s}")
            nc.sync.dma_start(out=t[:], in_=sv[s][:])
            sv_sb.append(t)

        gbuf = [None, None]

        def fetch_chunk(s, ci):
            ge = cfg.gelem
            # trim the final call to the tiles actually consumed
            n = min(CH, plan.t_stream(s) - ci * CH)
            gbuf[s] = gpool[s].tile([128, CH * max(D, ge)], bf16, tag=f"gb{s}",
                                    name=f"gbuf{s}")
            nc.gpsimd.dma_gather(
                gbuf[s][:, :n * ge].rearrange("p (k j) -> p k j", j=ge),
                xp[s][:] if ge == D
                else (xp[s].rearrange("a (c d) -> (a c) d", d=ge) if ge < D
                      else xp[s].rearrange("(a c) d -> a (c d)", c=ge // D)),
                gi_sb[s][:, ci * CH * 8:ci * CH * 8 + n * 8],
                n * 128, n * 128, ge,
                single_packet=(CH * 128 <= 1024),
                queue_num=(s * 2 + ci % 2 if cfg.gather_queues == 4
                           else s % cfg.gather_queues),
            )

        def body():
            if cfg.skip_compute:
                for ci in range(max(tp) // CH):
                    for s in range(2):
                        if ci < tp[s] // CH:
                            fetch_chunk(s, ci)
                return
            spos = [0, 0]
            for g in range(NG):
                rows_g = min(GR, RPC - g * GR)
                agg = ps_agg.tile([128, GR], f32)
                first = True
                for t in range(max(plan.tc[g])):
                    for s in range(2):
                        if t >= plan.tc[g][s]:
                            continue
                        if spos[s] % CH == 0:
                            fetch_chunk(s, spos[s] // CH)
                        k = spos[s] % CH
                        base = plan.db[g][s][t]
                        nc.tensor.matmul(
                            out=agg[:, base:base + BW],
                            lhsT=gbuf[s][:, k * D:(k + 1) * D],
                            rhs=sv_sb[s][:, spos[s] * BW:(spos[s] + 1) * BW],
                            start=first, stop=True,
                            skip_group_check=True,
                        )
                        first = False
                        spos[s] += 1
                agg_sb = epool.tile([128, GR], f32, tag="aggsb")
                nc.vector.tensor_copy(out=agg_sb[:, :rows_g],
                                      in_=agg[:, :rows_g])
                for rc in range(math.ceil(rows_g / 128)):
                    w = min(128, rows_g - rc * 128)
                    out_ps = ps_out.tile([128, D], f32)
                    nc.tensor.matmul(
                        out=out_ps[:w, :],
                        lhsT=agg_sb[:, rc * 128:rc * 128 + w],
                        rhs=wt_ap(),
                        start=True, stop=True,
                    )
                    out_sb = epool.tile([128, D], f32, tag="outsb")
                    nc.vector.tensor_tensor(
                        out=out_sb[:w, :], in0=out_ps[:w, :], in1=bb_ap(w),
                        op=mybir.AluOpType.add,
                    )
                    r0 = g * GR + rc * 128
                    nc.sync.dma_start(out=y[r0:r0 + w, :], in_=out_sb[:w, :])

        if cfg.loop_n > 0:
            with tc.For_i(0, cfg.loop_n, 1):
                body()
        else:
            for _ in range(cfg.repeats):
                body()

    nc.compile()
    return nc


_CACHE = {}


def _get_nc(cfg: Cfg, plan: Plan):
    key = (cfg, plan)
    if key not in _CACHE:
        _CACHE[key] = _build_nc(cfg, plan)
    return _CACHE[key]


def kernel(x, edge_row, edge_col, edge_val, W, b):
    from concourse.bass_utils import run_bass_kernel_spmd

    cfg = Cfg()
    plan, in_maps = _preprocess(cfg, x, edge_row, edge_col, edge_val, W, b)
    nc = _get_nc(cfg, plan)
    res = run_bass_kernel_spmd(nc, in_maps, core_ids=list(range(cfg.n_cores)))
    RPC = cfg.rows_per_core
    out = np.empty((cfg.n_nodes, D), dtype=np.float32)
    for c in range(cfg.n_cores):
        out[c * RPC:(c + 1) * RPC] = res.results[c]["y"][:RPC]
    return out



# revision 24
# speedup vs baseline: 1.0359x; 1.0359x over previous
"""GCN layer (SpMM + linear) on 8 Trainium2 NeuronCores — exact-packed dest-banded tiles, host-dense selection (bf16).

out[i] = (sum_{e: edge_row[e]==i} edge_val[e] * x[edge_col[e]]) @ W.T + b

Destination rows are partitioned across 8 cores (6250 each) into 13 PSUM
groups of 496 rows.  Per (group, source-half) bucket, edges are sorted by
destination and packed 128 per gather tile — no per-window padding.  Tile t's
destinations fall in a narrow data-derived band [db[t], db[t]+BW); the
selection matrix sval[slot, dest-db] (bf16, host-precomputed, resident in
SBUF) is dense over the band, so duplicate (src,dst) edges just sum.

Each slot gathers one 256B bf16 x row via SWDGE dma_gather (int16 indices,
lo/hi source halves on separate queues).  matmul(lhsT=gathered, rhs=sval
band) accumulates agg.T[feat, dest] into the group's PSUM bank.  Epilogue per
group: copy to SBUF, project with W.T (fp32), add bias, DMA out.
"""

import math
from dataclasses import dataclass

import numpy as np

GR = 496          # dest rows per PSUM group
CAP = 128         # slots per gather tile
D = 128           # feature dim


@dataclass(frozen=True)
class Cfg:
    n_nodes: int = 50000
    n_edges: int = 800000
    n_cores: int = 8
    chunk: int = 32           # tiles per dma_gather call
    gather_queues: int = 4
    gather_bufs: int = 6
    agg_bufs: int = 2         # PSUM agg groups in flight
    gelem: int = 128          # gather elems per descriptor (diagnostic only)
    dma_scratch: int = 16384  # SWDGE descriptor ring carveout bytes/partition
    epi_delay: int = 2        # groups the projection trails the aggregation by
    proj_bf16: bool = True    # project in bf16 (agg cast during PSUM drain)
    bucket_calls: bool = False  # one dma_gather per (group, stream) bucket
    trim: bool = False          # runtime-trim gathers to per-core edge counts
    # (bucket_calls+trim cuts ~4% of gather descriptors but the register-length
    #  path wedges the device on this runtime -- left off)
    skip_compute: bool = False
    skip_gather: bool = False   # ablation: matmuls on stale buffers
    skip_epilog: bool = False   # ablation: no PSUM drain / projection / store
    loop_n: int = 0
    repeats: int = 1

    @property
    def split(self) -> int:
        return self.n_nodes // 2

    @property
    def rows_per_core(self) -> int:
        return self.n_nodes // self.n_cores

    @property
    def n_groups(self) -> int:
        return math.ceil(self.rows_per_core / GR)


@dataclass(frozen=True)
class Plan:
    tc: tuple    # tc[g][s]: tiles per (group, stream), max over cores
    db: tuple    # db[g][s][t]: band start column of tile t
    bw: int      # band width (compiled free size of the sval matmul)

    def t_stream(self, s):
        return sum(t[s] for t in self.tc)


def _preprocess(cfg: Cfg, x, edge_row, edge_col, edge_val, W, b):
    import ml_dtypes

    RPC = cfg.rows_per_core
    NG = cfg.n_groups
    SPLIT = cfg.split

    x = np.asarray(x)
    edge_row = np.asarray(edge_row)
    edge_col = np.asarray(edge_col)
    edge_val = np.asarray(edge_val)

    xb = np.ascontiguousarray(x.astype(ml_dtypes.bfloat16))
    xplo = np.ascontiguousarray(xb[:SPLIT])
    xphi = np.ascontiguousarray(xb[SPLIT:])

    # Pass 1: per (core, group, stream) dest-sorted edge arrays.
    per_core = []
    for c in range(cfg.n_cores):
        e0, e1 = np.searchsorted(edge_row, [c * RPC, (c + 1) * RPC])
        r_loc = edge_row[e0:e1] - c * RPC
        g_of_e = r_loc // GR
        d_loc = r_loc - g_of_e * GR
        src = edge_col[e0:e1].astype(np.int64)
        s_of_e = (src >= SPLIT).astype(np.int64)
        val = edge_val[e0:e1].astype(np.float64)
        buckets = {}
        for g in range(NG):
            for s in range(2):
                m = (g_of_e == g) & (s_of_e == s)
                order = np.argsort(d_loc[m], kind="stable")
                buckets[(g, s)] = (src[m][order] - s * SPLIT,
                                  d_loc[m][order], val[m][order])
        per_core.append(buckets)

    # Plan: tile counts, band starts, band width (shared across cores).
    tc = []
    for g in range(NG):
        tc.append(tuple(
            max(math.ceil(len(pc[(g, s)][0]) / CAP) for pc in per_core)
            for s in range(2)))

    db = []
    bw = 0
    for g in range(NG):
        row = []
        for s in range(2):
            nt = tc[g][s]
            starts = np.full(nt, GR, dtype=np.int64)
            ends = np.zeros(nt, dtype=np.int64)
            for pc in per_core:
                dd = pc[(g, s)][1]
                for t in range(nt):
                    seg = dd[t * CAP:(t + 1) * CAP]
                    if len(seg):
                        starts[t] = min(starts[t], seg[0])
                        ends[t] = max(ends[t], seg[-1] + 1)
            starts = np.minimum(starts, ends)  # empty tiles -> band at end
            bw = max(bw, int((ends - starts).max(initial=0)))
            row.append(tuple(int(v) for v in starts))
        db.append(tuple(row))
    bw = math.ceil(bw / 16) * 16
    # clamp band starts so db+bw stays inside the group
    db = tuple(
        tuple(tuple(min(v, GR - bw) for v in row_s) for row_s in row)
        for row in db)
    plan = Plan(tc=tuple(tc), db=db, bw=bw)

    CH = cfg.chunk
    tp = [math.ceil(plan.t_stream(s) / CH) * CH for s in range(2)]

    wt = np.asarray(W).T.astype(
        ml_dtypes.bfloat16 if cfg.proj_bf16 else np.float32)
    bb = np.tile(np.asarray(b)[None, :].astype(np.float32), (128, 1))

    def wrap_idx(idx_lin):
        return np.tile(np.ascontiguousarray(idx_lin.reshape(-1, 16).T), (8, 1))

    in_maps = []
    for c in range(cfg.n_cores):
        buckets = per_core[c]
        idx_lin = [np.zeros(tp[s] * CAP, dtype=np.int16) for s in range(2)]
        sval = [np.zeros((128, tp[s] * bw), dtype=np.float64) for s in range(2)]
        spos = [0, 0]
        for g in range(NG):
            for t in range(max(plan.tc[g])):
                for s in range(2):
                    if t >= plan.tc[g][s]:
                        continue
                    ss, dd, vv = buckets[(g, s)]
                    ss = ss[t * CAP:(t + 1) * CAP]
                    dd = dd[t * CAP:(t + 1) * CAP]
                    vv = vv[t * CAP:(t + 1) * CAP]
                    pos = spos[s]
                    base = plan.db[g][s][t]
                    idx_lin[s][pos * CAP:pos * CAP + len(ss)] = ss
                    np.add.at(sval[s],
                              (np.arange(len(ss)), pos * bw + dd - base), vv)
                    spos[s] += 1
        nidx = []
        start = [0, 0]
        for g in range(NG):
            for s in range(2):
                n = len(buckets[(g, s)][0])
                nv = min(math.ceil(n / 16) * 16, plan.tc[g][s] * CAP)
                nidx.append(nv)
                if cfg.trim:
                    # gather contract: num_idxs_reg == count of indices >= 0,
                    # negatives must trail -- mark trimmed pad slots with -1
                    b0 = start[s] * CAP
                    idx_lin[s][b0 + nv:b0 + plan.tc[g][s] * CAP] = -1
                start[s] += plan.tc[g][s]
        in_maps.append({
            "xplo": xplo, "xphi": xphi,
            "nidx": np.ascontiguousarray(
                np.asarray(nidx, dtype=np.int32)[None, :]),
            "gilo": np.ascontiguousarray(wrap_idx(idx_lin[0])),
            "gihi": np.ascontiguousarray(wrap_idx(idx_lin[1])),
            "svlo": np.ascontiguousarray(
                sval[0].astype(np.float32).astype(
                    np.asarray(xb).dtype)),
            "svhi": np.ascontiguousarray(
                sval[1].astype(np.float32).astype(
                    np.asarray(xb).dtype)),
            "cst": np.ascontiguousarray(bb, dtype=np.float32),
            "wt": np.ascontiguousarray(wt),
        })
    return plan, in_maps


def _build_nc(cfg: Cfg, plan: Plan):
    from contextlib import ExitStack

    import concourse.bacc as bacc
    import concourse.mybir as mybir
    import concourse.tile as tile

    f32 = mybir.dt.float32
    bf16 = mybir.dt.bfloat16
    i16 = mybir.dt.int16
    NG = cfg.n_groups
    RPC = cfg.rows_per_core
    CH = cfg.chunk
    SPLIT = cfg.split
    BW = plan.bw
    tp = [math.ceil(plan.t_stream(s) / CH) * CH for s in range(2)]
    pdt_np = bf16 if cfg.proj_bf16 else f32

    nc = bacc.Bacc("TRN2", target_bir_lowering=False,
                   num_swdge_queues=cfg.gather_queues,
                   dynamic_dma_scratch_size=cfg.dma_scratch)
    xp = [nc.dram_tensor("xplo", [SPLIT, D], bf16, kind="ExternalInput"),
          nc.dram_tensor("xphi", [SPLIT, D], bf16, kind="ExternalInput")]
    gi = [nc.dram_tensor("gilo", [128, tp[0] * 8], i16, kind="ExternalInput"),
          nc.dram_tensor("gihi", [128, tp[1] * 8], i16, kind="ExternalInput")]
    sv = [nc.dram_tensor("svlo", [128, tp[0] * BW], bf16,
                         kind="ExternalInput"),
          nc.dram_tensor("svhi", [128, tp[1] * BW], bf16,
                         kind="ExternalInput")]
    cst = nc.dram_tensor("cst", [128, D], f32, kind="ExternalInput")
    wtd = nc.dram_tensor("wt", [128, D], pdt_np, kind="ExternalInput")
    i32 = mybir.dt.int32
    NCALLS = 2 * NG
    nidx_d = nc.dram_tensor("nidx", [1, NCALLS], i32, kind="ExternalInput")
    y = nc.dram_tensor("y", [NG * GR, D], f32, kind="ExternalOutput")
    TCMAX = max(max(t) for t in plan.tc)

    with tile.TileContext(nc) as tc, ExitStack() as ctx:
        const = ctx.enter_context(tc.tile_pool(name="const", bufs=1))
        gpool = [ctx.enter_context(tc.tile_pool(name="glo",
                                                bufs=cfg.gather_bufs)),
                 ctx.enter_context(tc.tile_pool(name="ghi",
                                                bufs=cfg.gather_bufs))]
        epool = ctx.enter_context(tc.tile_pool(name="epilog",
                                               bufs=cfg.epi_delay + 1))
        opool = ctx.enter_context(tc.tile_pool(name="outp", bufs=4))
        ps_agg = ctx.enter_context(tc.tile_pool(name="psagg", bufs=cfg.agg_bufs,
                                                space="PSUM"))
        ps_out = ctx.enter_context(tc.tile_pool(name="psout", bufs=4,
                                                space="PSUM"))

        cst_sb = const.tile([128, D], f32)
        nc.sync.dma_start(out=cst_sb[:], in_=cst[:])
        wt_sb = const.tile([128, D], pdt_np, tag="wt")
        nc.sync.dma_start(out=wt_sb[:], in_=wtd[:])
        nidx_sb = const.tile([1, NCALLS], i32, tag="nidx")
        nc.sync.dma_start(out=nidx_sb[:], in_=nidx_d[:])

        def bb_ap(p):
            return cst_sb[:p, :]

        def wt_ap():
            return wt_sb[:, :]

        gi_sb = []
        sv_sb = []
        for s in range(2):
            t = const.tile([128, tp[s] * 8], i16, tag=f"gi{s}")
            nc.sync.dma_start(out=t[:], in_=gi[s][:])
            gi_sb.append(t)
            t = const.tile([128, tp[s] * BW], bf16, tag=f"sv{s}")
            nc.sync.dma_start(out=t[:], in_=sv[s][:])
            sv_sb.append(t)

        GW = (TCMAX if cfg.bucket_calls else CH) * max(D, cfg.gelem)
        nregs = None
        if cfg.bucket_calls and cfg.trim:
            # hoisted: per-core gather lengths, constant across repeats
            nregs = [
                nc.gpsimd.value_load(
                    nidx_sb[:1, 2 * g + s:2 * g + s + 1],
                    min_val=0, max_val=plan.tc[g][s] * CAP)
                for g in range(NG) for s in range(2)]
            nregs = {2 * g + s: nregs[2 * g + s]
                     for g in range(NG) for s in range(2)}
        gbuf = [None, None]
        if cfg.skip_gather or (cfg.bucket_calls and cfg.trim):
            # memset every physical pool slot once so trimmed tail slots never
            # expose uninitialized SBUF (NaN * 0 = NaN in the PSUM accumulate)
            for s in range(2):
                for _ in range(1 if cfg.skip_gather else cfg.gather_bufs):
                    gbuf[s] = gpool[s].tile([128, GW], bf16, tag=f"gb{s}",
                                            name=f"gbuf{s}")
                    nc.vector.memset(gbuf[s][:], 0.0)

        def fetch_chunk(s, ci):
            ge = cfg.gelem
            # trim the final call to the tiles actually consumed
            n = min(CH, plan.t_stream(s) - ci * CH)
            gbuf[s] = gpool[s].tile([128, GW], bf16, tag=f"gb{s}",
                                    name=f"gbuf{s}")
            nc.gpsimd.dma_gather(
                gbuf[s][:, :n * ge].rearrange("p (k j) -> p k j", j=ge),
                xp[s][:] if ge == D
                else (xp[s].rearrange("a (c d) -> (a c) d", d=ge) if ge < D
                      else xp[s].rearrange("(a c) d -> a (c d)", c=ge // D)),
                gi_sb[s][:, ci * CH * 8:ci * CH * 8 + n * 8],
                n * 128, n * 128, ge,
                single_packet=(CH * 128 <= 1024),
                queue_num=(s * 2 + ci % 2 if cfg.gather_queues == 4
                           else s % cfg.gather_queues),
            )

        def fetch_bucket(s, g, spos0):
            ge = cfg.gelem
            ci = 2 * g + s
            nt = plan.tc[g][s]
            gbuf[s] = gpool[s].tile([128, GW], bf16, tag=f"gb{s}",
                                    name=f"gbuf{s}")
            nreg = nregs[ci] if cfg.trim else nt * 128
            nc.gpsimd.dma_gather(
                gbuf[s][:, :nt * ge].rearrange("p (k j) -> p k j", j=ge),
                xp[s][:],
                gi_sb[s][:, spos0 * 8:(spos0 + nt) * 8],
                nt * 128, nreg, ge,
                single_packet=(nt * 128 <= 1024),
                queue_num=(s * 2 + g % 2 if cfg.gather_queues == 4
                           else s % cfg.gather_queues),
            )

        def body():
            if cfg.skip_compute:
                if cfg.bucket_calls:
                    sp = [0, 0]
                    for g in range(NG):
                        for s in range(2):
                            fetch_bucket(s, g, sp[s])
                            sp[s] += plan.tc[g][s]
                else:
                    for ci in range(max(tp) // CH):
                        for s in range(2):
                            if ci < tp[s] // CH:
                                fetch_chunk(s, ci)
                return
            spos = [0, 0]
            pending = []

            def project(g, agg_sb, rows_g):
                for rc in range(math.ceil(rows_g / 128)):
                    w = min(128, rows_g - rc * 128)
                    out_ps = ps_out.tile([128, D], f32)
                    nc.tensor.matmul(
                        out=out_ps[:w, :],
                        lhsT=agg_sb[:, rc * 128:rc * 128 + w],
                        rhs=wt_ap(),
                        start=True, stop=True,
                    )
                    out_sb = opool.tile([128, D], f32, tag="outsb")
                    nc.vector.tensor_tensor(
                        out=out_sb[:w, :], in0=out_ps[:w, :], in1=bb_ap(w),
                        op=mybir.AluOpType.add,
                    )
                    r0 = g * GR + rc * 128
                    nc.sync.dma_start(out=y[r0:r0 + w, :], in_=out_sb[:w, :])

            for g in range(NG):
                rows_g = min(GR, RPC - g * GR)
                agg = ps_agg.tile([128, GR], f32)
                first = True
                for t in range(max(plan.tc[g])):
                    for s in range(2):
                        if t >= plan.tc[g][s]:
                            continue
                        if not cfg.skip_gather:
                            if cfg.bucket_calls:
                                if t == 0:
                                    fetch_bucket(s, g, spos[s])
                            elif spos[s] % CH == 0:
                                fetch_chunk(s, spos[s] // CH)
                        k = t if cfg.bucket_calls else spos[s] % CH
                        base = plan.db[g][s][t]
                        nc.tensor.matmul(
                            out=agg[:, base:base + BW],
                            lhsT=gbuf[s][:, k * D:(k + 1) * D],
                            rhs=sv_sb[s][:, spos[s] * BW:(spos[s] + 1) * BW],
                            start=first, stop=True,
                            skip_group_check=True,
                        )
                        first = False
                        spos[s] += 1
                if cfg.skip_epilog:
                    continue
                # projection for group g-epi_delay: its PSUM drain finished
                # groups ago, so the PE does not stall on the DVE here
                while pending and pending[0][0] <= g - cfg.epi_delay:
                    project(*pending.pop(0))
                agg_sb = epool.tile([128, GR], pdt_np, tag="aggsb")
                nc.vector.tensor_copy(out=agg_sb[:, :rows_g],
                                      in_=agg[:, :rows_g])
                pending.append((g, agg_sb, rows_g))
            for item in pending:
                project(*item)

        if cfg.loop_n > 0:
            with tc.For_i(0, cfg.loop_n, 1):
                body()
        else:
            for _ in range(cfg.repeats):
                body()

    nc.compile()
    return nc


_CACHE = {}


def _get_nc(cfg: Cfg, plan: Plan):
    key = (cfg, plan)
    if key not in _CACHE:
        _CACHE[key] = _build_nc(cfg, plan)
    return _CACHE[key]


def kernel(x, edge_row, edge_col, edge_val, W, b):
    from concourse.bass_utils import run_bass_kernel_spmd

    cfg = Cfg()
    plan, in_maps = _preprocess(cfg, x, edge_row, edge_col, edge_val, W, b)
    nc = _get_nc(cfg, plan)
    res = run_bass_kernel_spmd(nc, in_maps, core_ids=list(range(cfg.n_cores)))
    RPC = cfg.rows_per_core
    out = np.empty((cfg.n_nodes, D), dtype=np.float32)
    for c in range(cfg.n_cores):
        out[c * RPC:(c + 1) * RPC] = res.results[c]["y"][:RPC]
    return out



# revision 25
# speedup vs baseline: 1.0525x; 1.0160x over previous
"""GCN layer (SpMM + linear) on 8 Trainium2 NeuronCores — exact-packed dest-banded tiles, host-dense selection (bf16).

out[i] = (sum_{e: edge_row[e]==i} edge_val[e] * x[edge_col[e]]) @ W.T + b

Destination rows are partitioned across 8 cores (6250 each) into 13 PSUM
groups of 496 rows.  Per (group, source-half) bucket, edges are sorted by
destination and packed 128 per gather tile — no per-window padding.  Tile t's
destinations fall in a narrow data-derived band [db[t], db[t]+BW); the
selection matrix sval[slot, dest-db] (bf16, host-precomputed, resident in
SBUF) is dense over the band, so duplicate (src,dst) edges just sum.

Each slot gathers one 256B bf16 x row via SWDGE dma_gather (int16 indices,
lo/hi source halves on separate queues), 32 tiles per call to amortize the
~1-2us/call descriptor-generation fixed cost.  matmul(lhsT=gathered, rhs=sval
band) accumulates agg.T[feat, dest] into the group's PSUM bank.  The epilogue
(PSUM drain -> bf16 projection with W.T -> bias add -> DMA out) is
software-pipelined two groups behind the aggregation so the in-order PE queue
never stalls on the DVE's PSUM drain.
"""

import math
from dataclasses import dataclass

import numpy as np

GR = 496          # dest rows per PSUM group
CAP = 128         # slots per gather tile
D = 128           # feature dim


@dataclass(frozen=True)
class Cfg:
    n_nodes: int = 50000
    n_edges: int = 800000
    n_cores: int = 8
    chunk: int = 32           # tiles per dma_gather call
    gather_queues: int = 4
    gather_bufs: int = 6
    agg_bufs: int = 2         # PSUM agg groups in flight
    gelem: int = 128          # gather elems per descriptor (diagnostic only)
    dma_scratch: int = 16384  # SWDGE descriptor ring carveout bytes/partition
    epi_delay: int = 2        # groups the projection trails the aggregation by
    proj_bf16: bool = True    # project in bf16 (agg cast during PSUM drain)
    bucket_calls: bool = False  # one dma_gather per (group, stream) bucket
    trim: bool = False          # runtime-trim gathers to per-core edge counts
    # (bucket_calls+trim cuts ~4% of gather descriptors but the register-length
    #  path wedges the device on this runtime -- left off)
    skip_compute: bool = False
    skip_gather: bool = False   # ablation: matmuls on stale buffers
    skip_epilog: bool = False   # ablation: no PSUM drain / projection / store
    loop_n: int = 0
    repeats: int = 1

    @property
    def split(self) -> int:
        return self.n_nodes // 2

    @property
    def rows_per_core(self) -> int:
        return self.n_nodes // self.n_cores

    @property
    def n_groups(self) -> int:
        return math.ceil(self.rows_per_core / GR)


@dataclass(frozen=True)
class Plan:
    tc: tuple    # tc[g][s]: tiles per (group, stream), max over cores
    db: tuple    # db[g][s][t]: band start column of tile t
    bw: int      # band width (compiled free size of the sval matmul)

    def t_stream(self, s):
        return sum(t[s] for t in self.tc)


def _preprocess(cfg: Cfg, x, edge_row, edge_col, edge_val, W, b):
    import ml_dtypes

    RPC = cfg.rows_per_core
    NG = cfg.n_groups
    SPLIT = cfg.split

    x = np.asarray(x)
    edge_row = np.asarray(edge_row)
    edge_col = np.asarray(edge_col)
    edge_val = np.asarray(edge_val)

    xb = np.ascontiguousarray(x.astype(ml_dtypes.bfloat16))
    xplo = np.ascontiguousarray(xb[:SPLIT])
    xphi = np.ascontiguousarray(xb[SPLIT:])

    # Pass 1: per (core, group, stream) dest-sorted edge arrays.
    per_core = []
    for c in range(cfg.n_cores):
        e0, e1 = np.searchsorted(edge_row, [c * RPC, (c + 1) * RPC])
        r_loc = edge_row[e0:e1] - c * RPC
        g_of_e = r_loc // GR
        d_loc = r_loc - g_of_e * GR
        src = edge_col[e0:e1].astype(np.int64)
        s_of_e = (src >= SPLIT).astype(np.int64)
        val = edge_val[e0:e1].astype(np.float64)
        buckets = {}
        for g in range(NG):
            for s in range(2):
                m = (g_of_e == g) & (s_of_e == s)
                order = np.argsort(d_loc[m], kind="stable")
                buckets[(g, s)] = (src[m][order] - s * SPLIT,
                                  d_loc[m][order], val[m][order])
        per_core.append(buckets)

    # Plan: tile counts, band starts, band width (shared across cores).
    tc = []
    for g in range(NG):
        tc.append(tuple(
            max(math.ceil(len(pc[(g, s)][0]) / CAP) for pc in per_core)
            for s in range(2)))

    db = []
    bw = 0
    for g in range(NG):
        row = []
        for s in range(2):
            nt = tc[g][s]
            starts = np.full(nt, GR, dtype=np.int64)
            ends = np.zeros(nt, dtype=np.int64)
            for pc in per_core:
                dd = pc[(g, s)][1]
                for t in range(nt):
                    seg = dd[t * CAP:(t + 1) * CAP]
                    if len(seg):
                        starts[t] = min(starts[t], seg[0])
                        ends[t] = max(ends[t], seg[-1] + 1)
            starts = np.minimum(starts, ends)  # empty tiles -> band at end
            bw = max(bw, int((ends - starts).max(initial=0)))
            row.append(tuple(int(v) for v in starts))
        db.append(tuple(row))
    bw = math.ceil(bw / 16) * 16
    # clamp band starts so db+bw stays inside the group
    db = tuple(
        tuple(tuple(min(v, GR - bw) for v in row_s) for row_s in row)
        for row in db)
    plan = Plan(tc=tuple(tc), db=db, bw=bw)

    CH = cfg.chunk
    tp = [math.ceil(plan.t_stream(s) / CH) * CH for s in range(2)]

    wt = np.asarray(W).T.astype(
        ml_dtypes.bfloat16 if cfg.proj_bf16 else np.float32)
    bb = np.tile(np.asarray(b)[None, :].astype(np.float32), (128, 1))

    def wrap_idx(idx_lin):
        return np.tile(np.ascontiguousarray(idx_lin.reshape(-1, 16).T), (8, 1))

    in_maps = []
    for c in range(cfg.n_cores):
        buckets = per_core[c]
        idx_lin = [np.zeros(tp[s] * CAP, dtype=np.int16) for s in range(2)]
        sval = [np.zeros((128, tp[s] * bw), dtype=np.float64) for s in range(2)]
        spos = [0, 0]
        for g in range(NG):
            for t in range(max(plan.tc[g])):
                for s in range(2):
                    if t >= plan.tc[g][s]:
                        continue
                    ss, dd, vv = buckets[(g, s)]
                    ss = ss[t * CAP:(t + 1) * CAP]
                    dd = dd[t * CAP:(t + 1) * CAP]
                    vv = vv[t * CAP:(t + 1) * CAP]
                    pos = spos[s]
                    base = plan.db[g][s][t]
                    idx_lin[s][pos * CAP:pos * CAP + len(ss)] = ss
                    np.add.at(sval[s],
                              (np.arange(len(ss)), pos * bw + dd - base), vv)
                    spos[s] += 1
        nidx = []
        start = [0, 0]
        for g in range(NG):
            for s in range(2):
                n = len(buckets[(g, s)][0])
                nv = min(math.ceil(n / 16) * 16, plan.tc[g][s] * CAP)
                nidx.append(nv)
                if cfg.trim:
                    # gather contract: num_idxs_reg == count of indices >= 0,
                    # negatives must trail -- mark trimmed pad slots with -1
                    b0 = start[s] * CAP
                    idx_lin[s][b0 + nv:b0 + plan.tc[g][s] * CAP] = -1
                start[s] += plan.tc[g][s]
        in_maps.append({
            "xplo": xplo, "xphi": xphi,
            "nidx": np.ascontiguousarray(
                np.asarray(nidx, dtype=np.int32)[None, :]),
            "gilo": np.ascontiguousarray(wrap_idx(idx_lin[0])),
            "gihi": np.ascontiguousarray(wrap_idx(idx_lin[1])),
            "svlo": np.ascontiguousarray(
                sval[0].astype(np.float32).astype(
                    np.asarray(xb).dtype)),
            "svhi": np.ascontiguousarray(
                sval[1].astype(np.float32).astype(
                    np.asarray(xb).dtype)),
            "cst": np.ascontiguousarray(bb, dtype=np.float32),
            "wt": np.ascontiguousarray(wt),
        })
    return plan, in_maps


def _build_nc(cfg: Cfg, plan: Plan):
    from contextlib import ExitStack

    import concourse.bacc as bacc
    import concourse.mybir as mybir
    import concourse.tile as tile

    f32 = mybir.dt.float32
    bf16 = mybir.dt.bfloat16
    i16 = mybir.dt.int16
    NG = cfg.n_groups
    RPC = cfg.rows_per_core
    CH = cfg.chunk
    SPLIT = cfg.split
    BW = plan.bw
    tp = [math.ceil(plan.t_stream(s) / CH) * CH for s in range(2)]
    pdt_np = bf16 if cfg.proj_bf16 else f32

    nc = bacc.Bacc("TRN2", target_bir_lowering=False,
                   num_swdge_queues=cfg.gather_queues,
                   dynamic_dma_scratch_size=cfg.dma_scratch)
    xp = [nc.dram_tensor("xplo", [SPLIT, D], bf16, kind="ExternalInput"),
          nc.dram_tensor("xphi", [SPLIT, D], bf16, kind="ExternalInput")]
    gi = [nc.dram_tensor("gilo", [128, tp[0] * 8], i16, kind="ExternalInput"),
          nc.dram_tensor("gihi", [128, tp[1] * 8], i16, kind="ExternalInput")]
    sv = [nc.dram_tensor("svlo", [128, tp[0] * BW], bf16,
                         kind="ExternalInput"),
          nc.dram_tensor("svhi", [128, tp[1] * BW], bf16,
                         kind="ExternalInput")]
    cst = nc.dram_tensor("cst", [128, D], f32, kind="ExternalInput")
    wtd = nc.dram_tensor("wt", [128, D], pdt_np, kind="ExternalInput")
    i32 = mybir.dt.int32
    NCALLS = 2 * NG
    nidx_d = nc.dram_tensor("nidx", [1, NCALLS], i32, kind="ExternalInput")
    y = nc.dram_tensor("y", [NG * GR, D], f32, kind="ExternalOutput")
    TCMAX = max(max(t) for t in plan.tc)

    with tile.TileContext(nc) as tc, ExitStack() as ctx:
        const = ctx.enter_context(tc.tile_pool(name="const", bufs=1))
        gpool = [ctx.enter_context(tc.tile_pool(name="glo",
                                                bufs=cfg.gather_bufs)),
                 ctx.enter_context(tc.tile_pool(name="ghi",
                                                bufs=cfg.gather_bufs))]
        epool = ctx.enter_context(tc.tile_pool(name="epilog",
                                               bufs=cfg.epi_delay + 1))
        opool = ctx.enter_context(tc.tile_pool(name="outp", bufs=4))
        ps_agg = ctx.enter_context(tc.tile_pool(name="psagg", bufs=cfg.agg_bufs,
                                                space="PSUM"))
        ps_out = ctx.enter_context(tc.tile_pool(name="psout", bufs=4,
                                                space="PSUM"))

        cst_sb = const.tile([128, D], f32)
        nc.sync.dma_start(out=cst_sb[:], in_=cst[:])
        wt_sb = const.tile([128, D], pdt_np, tag="wt")
        nc.sync.dma_start(out=wt_sb[:], in_=wtd[:])
        nidx_sb = const.tile([1, NCALLS], i32, tag="nidx")
        nc.sync.dma_start(out=nidx_sb[:], in_=nidx_d[:])

        def bb_ap(p):
            return cst_sb[:p, :]

        def wt_ap():
            return wt_sb[:, :]

        gi_sb = []
        sv_sb = []
        for s in range(2):
            t = const.tile([128, tp[s] * 8], i16, tag=f"gi{s}")
            nc.sync.dma_start(out=t[:], in_=gi[s][:])
            gi_sb.append(t)
            t = const.tile([128, tp[s] * BW], bf16, tag=f"sv{s}")
            nc.sync.dma_start(out=t[:], in_=sv[s][:])
            sv_sb.append(t)

        GW = (TCMAX if cfg.bucket_calls else CH) * max(D, cfg.gelem)
        nregs = None
        if cfg.bucket_calls and cfg.trim:
            # hoisted: per-core gather lengths, constant across repeats
            nregs = [
                nc.gpsimd.value_load(
                    nidx_sb[:1, 2 * g + s:2 * g + s + 1],
                    min_val=0, max_val=plan.tc[g][s] * CAP)
                for g in range(NG) for s in range(2)]
            nregs = {2 * g + s: nregs[2 * g + s]
                     for g in range(NG) for s in range(2)}
        gbuf = [None, None]
        if cfg.skip_gather or (cfg.bucket_calls and cfg.trim):
            # memset every physical pool slot once so trimmed tail slots never
            # expose uninitialized SBUF (NaN * 0 = NaN in the PSUM accumulate)
            for s in range(2):
                for _ in range(1 if cfg.skip_gather else cfg.gather_bufs):
                    gbuf[s] = gpool[s].tile([128, GW], bf16, tag=f"gb{s}",
                                            name=f"gbuf{s}")
                    nc.vector.memset(gbuf[s][:], 0.0)

        def fetch_chunk(s, ci):
            ge = cfg.gelem
            # trim the final call to the tiles actually consumed
            n = min(CH, plan.t_stream(s) - ci * CH)
            gbuf[s] = gpool[s].tile([128, GW], bf16, tag=f"gb{s}",
                                    name=f"gbuf{s}")
            nc.gpsimd.dma_gather(
                gbuf[s][:, :n * ge].rearrange("p (k j) -> p k j", j=ge),
                xp[s][:] if ge == D
                else (xp[s].rearrange("a (c d) -> (a c) d", d=ge) if ge < D
                      else xp[s].rearrange("(a c) d -> a (c d)", c=ge // D)),
                gi_sb[s][:, ci * CH * 8:ci * CH * 8 + n * 8],
                n * 128, n * 128, ge,
                single_packet=(CH * 128 <= 1024),
                queue_num=(s * 2 + ci % 2 if cfg.gather_queues == 4
                           else s % cfg.gather_queues),
            )

        def fetch_bucket(s, g, spos0):
            ge = cfg.gelem
            ci = 2 * g + s
            nt = plan.tc[g][s]
            gbuf[s] = gpool[s].tile([128, GW], bf16, tag=f"gb{s}",
                                    name=f"gbuf{s}")
            nreg = nregs[ci] if cfg.trim else nt * 128
            nc.gpsimd.dma_gather(
                gbuf[s][:, :nt * ge].rearrange("p (k j) -> p k j", j=ge),
                xp[s][:],
                gi_sb[s][:, spos0 * 8:(spos0 + nt) * 8],
                nt * 128, nreg, ge,
                single_packet=(nt * 128 <= 1024),
                queue_num=(s * 2 + g % 2 if cfg.gather_queues == 4
                           else s % cfg.gather_queues),
            )

        def body():
            if cfg.skip_compute:
                if cfg.bucket_calls:
                    sp = [0, 0]
                    for g in range(NG):
                        for s in range(2):
                            fetch_bucket(s, g, sp[s])
                            sp[s] += plan.tc[g][s]
                else:
                    for ci in range(max(tp) // CH):
                        for s in range(2):
                            if ci < tp[s] // CH:
                                fetch_chunk(s, ci)
                return
            spos = [0, 0]
            pending = []

            def project(g, agg_sb, rows_g):
                for rc in range(math.ceil(rows_g / 128)):
                    w = min(128, rows_g - rc * 128)
                    out_ps = ps_out.tile([128, D], f32)
                    nc.tensor.matmul(
                        out=out_ps[:w, :],
                        lhsT=agg_sb[:, rc * 128:rc * 128 + w],
                        rhs=wt_ap(),
                        start=True, stop=True,
                    )
                    out_sb = opool.tile([128, D], f32, tag="outsb")
                    nc.vector.tensor_tensor(
                        out=out_sb[:w, :], in0=out_ps[:w, :], in1=bb_ap(w),
                        op=mybir.AluOpType.add,
                    )
                    r0 = g * GR + rc * 128
                    nc.sync.dma_start(out=y[r0:r0 + w, :], in_=out_sb[:w, :])

            for g in range(NG):
                rows_g = min(GR, RPC - g * GR)
                agg = ps_agg.tile([128, GR], f32)
                first = True
                for t in range(max(plan.tc[g])):
                    for s in range(2):
                        if t >= plan.tc[g][s]:
                            continue
                        if not cfg.skip_gather:
                            if cfg.bucket_calls:
                                if t == 0:
                                    fetch_bucket(s, g, spos[s])
                            elif spos[s] % CH == 0:
                                fetch_chunk(s, spos[s] // CH)
                        k = t if cfg.bucket_calls else spos[s] % CH
                        base = plan.db[g][s][t]
                        nc.tensor.matmul(
                            out=agg[:, base:base + BW],
                            lhsT=gbuf[s][:, k * D:(k + 1) * D],
                            rhs=sv_sb[s][:, spos[s] * BW:(spos[s] + 1) * BW],
                            start=first, stop=True,
                            skip_group_check=True,
                        )
                        first = False
                        spos[s] += 1
                if cfg.skip_epilog:
                    continue
                # projection for group g-epi_delay: its PSUM drain finished
                # groups ago, so the PE does not stall on the DVE here
                while pending and pending[0][0] <= g - cfg.epi_delay:
                    project(*pending.pop(0))
                agg_sb = epool.tile([128, GR], pdt_np, tag="aggsb")
                nc.vector.tensor_copy(out=agg_sb[:, :rows_g],
                                      in_=agg[:, :rows_g])
                pending.append((g, agg_sb, rows_g))
            for item in pending:
                project(*item)

        if cfg.loop_n > 0:
            with tc.For_i(0, cfg.loop_n, 1):
                body()
        else:
            for _ in range(cfg.repeats):
                body()

    nc.compile()
    return nc


_CACHE = {}


def _get_nc(cfg: Cfg, plan: Plan):
    key = (cfg, plan)
    if key not in _CACHE:
        _CACHE[key] = _build_nc(cfg, plan)
    return _CACHE[key]


def kernel(x, edge_row, edge_col, edge_val, W, b):
    from concourse.bass_utils import run_bass_kernel_spmd

    cfg = Cfg()
    plan, in_maps = _preprocess(cfg, x, edge_row, edge_col, edge_val, W, b)
    nc = _get_nc(cfg, plan)
    res = run_bass_kernel_spmd(nc, in_maps, core_ids=list(range(cfg.n_cores)))
    RPC = cfg.rows_per_core
    out = np.empty((cfg.n_nodes, D), dtype=np.float32)
    for c in range(cfg.n_cores):
        out[c * RPC:(c + 1) * RPC] = res.results[c]["y"][:RPC]
    return out



# revision 26
# speedup vs baseline: 1.2405x; 1.1786x over previous
"""GCN layer (SpMM + linear) on 8 Trainium2 NeuronCores — exact-packed dest-banded tiles, host-dense selection (bf16).

out[i] = (sum_{e: edge_row[e]==i} edge_val[e] * x[edge_col[e]]) @ W.T + b

Destination rows are partitioned across 8 cores (6250 each) into 13 PSUM
groups of 496 rows.  Per (group, source-half) bucket, edges are sorted by
destination and packed 128 per gather tile — no per-window padding.  Tile t's
destinations fall in a narrow data-derived band [db[t], db[t]+BW); the
selection matrix sval[slot, dest-db] (bf16, host-precomputed, resident in
SBUF) is dense over the band, so duplicate (src,dst) edges just sum.

Each slot gathers one 256B bf16 x row via SWDGE dma_gather (int16 indices,
lo/hi source halves on separate queues), 32 tiles per call to amortize the
~1-2us/call descriptor-generation fixed cost.  matmul(lhsT=gathered, rhs=sval
band) accumulates agg.T[feat, dest] into the group's PSUM bank.  The epilogue
(PSUM drain -> bf16 projection with W.T -> bias add -> DMA out) is
software-pipelined two groups behind the aggregation so the in-order PE queue
never stalls on the DVE's PSUM drain.
"""

import math
from dataclasses import dataclass

import numpy as np

GR = 496          # dest rows per PSUM group
CAP = 128         # slots per gather tile
D = 128           # feature dim


@dataclass(frozen=True)
class Cfg:
    n_nodes: int = 50000
    n_edges: int = 800000
    n_cores: int = 8
    chunk: int = 32           # tiles per dma_gather call
    gather_queues: int = 4
    gather_bufs: int = 6
    agg_bufs: int = 2         # PSUM agg groups in flight
    gelem: int = 128          # gather elems per descriptor (diagnostic only)
    dma_scratch: int = 16384  # SWDGE descriptor ring carveout bytes/partition
    epi_delay: int = 2        # groups the projection trails the aggregation by
    proj_bf16: bool = True    # project in bf16 (agg cast during PSUM drain)
    bucket_calls: bool = False  # one dma_gather per (group, stream) bucket
    trim: bool = False          # runtime-trim gathers to per-core edge counts
    # (bucket_calls+trim cuts ~4% of gather descriptors but the register-length
    #  path wedges the device on this runtime -- left off)
    skip_compute: bool = False
    skip_gather: bool = False   # ablation: matmuls on stale buffers
    skip_epilog: bool = False   # ablation: no PSUM drain / projection / store
    loop_n: int = 0
    repeats: int = 1

    @property
    def split(self) -> int:
        return self.n_nodes // 2

    @property
    def rows_per_core(self) -> int:
        return self.n_nodes // self.n_cores

    @property
    def n_groups(self) -> int:
        return math.ceil(self.rows_per_core / GR)


@dataclass(frozen=True)
class Plan:
    tc: tuple    # tc[g][s]: tiles per (group, stream), max over cores
    db: tuple    # db[g][s][t]: band start column of tile t
    bw: int      # band width (compiled free size of the sval matmul)

    def t_stream(self, s):
        return sum(t[s] for t in self.tc)


def _preprocess(cfg: Cfg, x, edge_row, edge_col, edge_val, W, b):
    import ml_dtypes

    RPC = cfg.rows_per_core
    NG = cfg.n_groups
    SPLIT = cfg.split

    x = np.asarray(x)
    edge_row = np.asarray(edge_row)
    edge_col = np.asarray(edge_col)
    edge_val = np.asarray(edge_val)

    xb = np.ascontiguousarray(x.astype(ml_dtypes.bfloat16))
    xplo = np.ascontiguousarray(xb[:SPLIT])
    xphi = np.ascontiguousarray(xb[SPLIT:])

    # Pass 1: per (core, group, stream) dest-sorted edge arrays.
    per_core = []
    for c in range(cfg.n_cores):
        e0, e1 = np.searchsorted(edge_row, [c * RPC, (c + 1) * RPC])
        r_loc = edge_row[e0:e1] - c * RPC
        g_of_e = r_loc // GR
        d_loc = r_loc - g_of_e * GR
        src = edge_col[e0:e1].astype(np.int64)
        s_of_e = (src >= SPLIT).astype(np.int64)
        val = edge_val[e0:e1].astype(np.float64)
        buckets = {}
        for g in range(NG):
            for s in range(2):
                m = (g_of_e == g) & (s_of_e == s)
                order = np.argsort(d_loc[m], kind="stable")
                buckets[(g, s)] = (src[m][order] - s * SPLIT,
                                  d_loc[m][order], val[m][order])
        per_core.append(buckets)

    # Plan: tile counts, band starts, band width (shared across cores).
    tc = []
    for g in range(NG):
        tc.append(tuple(
            max(math.ceil(len(pc[(g, s)][0]) / CAP) for pc in per_core)
            for s in range(2)))

    db = []
    bw = 0
    for g in range(NG):
        row = []
        for s in range(2):
            nt = tc[g][s]
            starts = np.full(nt, GR, dtype=np.int64)
            ends = np.zeros(nt, dtype=np.int64)
            for pc in per_core:
                dd = pc[(g, s)][1]
                for t in range(nt):
                    seg = dd[t * CAP:(t + 1) * CAP]
                    if len(seg):
                        starts[t] = min(starts[t], seg[0])
                        ends[t] = max(ends[t], seg[-1] + 1)
            starts = np.minimum(starts, ends)  # empty tiles -> band at end
            bw = max(bw, int((ends - starts).max(initial=0)))
            row.append(tuple(int(v) for v in starts))
        db.append(tuple(row))
    bw = math.ceil(bw / 16) * 16
    # clamp band starts so db+bw stays inside the group
    db = tuple(
        tuple(tuple(min(v, GR - bw) for v in row_s) for row_s in row)
        for row in db)
    plan = Plan(tc=tuple(tc), db=db, bw=bw)

    CH = cfg.chunk
    tp = [math.ceil(plan.t_stream(s) / CH) * CH for s in range(2)]

    wt = np.asarray(W).T.astype(
        ml_dtypes.bfloat16 if cfg.proj_bf16 else np.float32)
    bb = np.tile(np.asarray(b)[None, :].astype(np.float32), (128, 1))

    def wrap_idx(idx_lin):
        return np.tile(np.ascontiguousarray(idx_lin.reshape(-1, 16).T), (8, 1))

    in_maps = []
    for c in range(cfg.n_cores):
        buckets = per_core[c]
        idx_lin = [np.zeros(tp[s] * CAP, dtype=np.int16) for s in range(2)]
        sval = [np.zeros((128, tp[s] * bw), dtype=np.float64) for s in range(2)]
        spos = [0, 0]
        for g in range(NG):
            for t in range(max(plan.tc[g])):
                for s in range(2):
                    if t >= plan.tc[g][s]:
                        continue
                    ss, dd, vv = buckets[(g, s)]
                    ss = ss[t * CAP:(t + 1) * CAP]
                    dd = dd[t * CAP:(t + 1) * CAP]
                    vv = vv[t * CAP:(t + 1) * CAP]
                    pos = spos[s]
                    base = plan.db[g][s][t]
                    idx_lin[s][pos * CAP:pos * CAP + len(ss)] = ss
                    np.add.at(sval[s],
                              (np.arange(len(ss)), pos * bw + dd - base), vv)
                    spos[s] += 1
        nidx = []
        start = [0, 0]
        for g in range(NG):
            for s in range(2):
                n = len(buckets[(g, s)][0])
                nv = min(math.ceil(n / 16) * 16, plan.tc[g][s] * CAP)
                nidx.append(nv)
                if cfg.trim:
                    # gather contract: num_idxs_reg == count of indices >= 0,
                    # negatives must trail -- mark trimmed pad slots with -1
                    b0 = start[s] * CAP
                    idx_lin[s][b0 + nv:b0 + plan.tc[g][s] * CAP] = -1
                start[s] += plan.tc[g][s]
        gdiv = max(1, cfg.gelem // 128)  # >1: timing-only 512B-desc experiment
        in_maps.append({
            "xplo": xplo, "xphi": xphi,
            "nidx": np.ascontiguousarray(
                np.asarray(nidx, dtype=np.int32)[None, :]),
            "gilo": np.ascontiguousarray(wrap_idx(idx_lin[0] // gdiv)),
            "gihi": np.ascontiguousarray(wrap_idx(idx_lin[1] // gdiv)),
            "svlo": np.ascontiguousarray(
                sval[0].astype(np.float32).astype(
                    np.asarray(xb).dtype)),
            "svhi": np.ascontiguousarray(
                sval[1].astype(np.float32).astype(
                    np.asarray(xb).dtype)),
            "cst": np.ascontiguousarray(bb, dtype=np.float32),
            "wt": np.ascontiguousarray(wt),
        })
    return plan, in_maps


def _build_nc(cfg: Cfg, plan: Plan):
    from contextlib import ExitStack

    import concourse.bacc as bacc
    import concourse.mybir as mybir
    import concourse.tile as tile

    f32 = mybir.dt.float32
    bf16 = mybir.dt.bfloat16
    i16 = mybir.dt.int16
    NG = cfg.n_groups
    RPC = cfg.rows_per_core
    CH = cfg.chunk
    SPLIT = cfg.split
    BW = plan.bw
    tp = [math.ceil(plan.t_stream(s) / CH) * CH for s in range(2)]
    pdt_np = bf16 if cfg.proj_bf16 else f32

    nc = bacc.Bacc("TRN2", target_bir_lowering=False,
                   num_swdge_queues=cfg.gather_queues,
                   dynamic_dma_scratch_size=cfg.dma_scratch)
    xp = [nc.dram_tensor("xplo", [SPLIT, D], bf16, kind="ExternalInput"),
          nc.dram_tensor("xphi", [SPLIT, D], bf16, kind="ExternalInput")]
    gi = [nc.dram_tensor("gilo", [128, tp[0] * 8], i16, kind="ExternalInput"),
          nc.dram_tensor("gihi", [128, tp[1] * 8], i16, kind="ExternalInput")]
    sv = [nc.dram_tensor("svlo", [128, tp[0] * BW], bf16,
                         kind="ExternalInput"),
          nc.dram_tensor("svhi", [128, tp[1] * BW], bf16,
                         kind="ExternalInput")]
    cst = nc.dram_tensor("cst", [128, D], f32, kind="ExternalInput")
    wtd = nc.dram_tensor("wt", [128, D], pdt_np, kind="ExternalInput")
    i32 = mybir.dt.int32
    NCALLS = 2 * NG
    nidx_d = nc.dram_tensor("nidx", [1, NCALLS], i32, kind="ExternalInput")
    y = nc.dram_tensor("y", [NG * GR, D], f32, kind="ExternalOutput")
    TCMAX = max(max(t) for t in plan.tc)

    with tile.TileContext(nc) as tc, ExitStack() as ctx:
        const = ctx.enter_context(tc.tile_pool(name="const", bufs=1))
        gpool = [ctx.enter_context(tc.tile_pool(name="glo",
                                                bufs=cfg.gather_bufs)),
                 ctx.enter_context(tc.tile_pool(name="ghi",
                                                bufs=cfg.gather_bufs))]
        epool = ctx.enter_context(tc.tile_pool(name="epilog",
                                               bufs=cfg.epi_delay + 1))
        opool = ctx.enter_context(tc.tile_pool(name="outp", bufs=4))
        ps_agg = ctx.enter_context(tc.tile_pool(name="psagg", bufs=cfg.agg_bufs,
                                                space="PSUM"))
        ps_out = ctx.enter_context(tc.tile_pool(name="psout", bufs=4,
                                                space="PSUM"))

        cst_sb = const.tile([128, D], f32)
        nc.sync.dma_start(out=cst_sb[:], in_=cst[:])
        wt_sb = const.tile([128, D], pdt_np, tag="wt")
        nc.sync.dma_start(out=wt_sb[:], in_=wtd[:])
        nidx_sb = const.tile([1, NCALLS], i32, tag="nidx")
        nc.sync.dma_start(out=nidx_sb[:], in_=nidx_d[:])

        def bb_ap(p):
            return cst_sb[:p, :]

        def wt_ap():
            return wt_sb[:, :]

        gi_sb = []
        sv_sb = []
        for s in range(2):
            t = const.tile([128, tp[s] * 8], i16, tag=f"gi{s}")
            nc.sync.dma_start(out=t[:], in_=gi[s][:])
            gi_sb.append(t)
            t = const.tile([128, tp[s] * BW], bf16, tag=f"sv{s}")
            nc.sync.dma_start(out=t[:], in_=sv[s][:])
            sv_sb.append(t)

        GW = (TCMAX if cfg.bucket_calls else CH) * max(D, cfg.gelem)
        nregs = None
        if cfg.bucket_calls and cfg.trim:
            # hoisted: per-core gather lengths, constant across repeats
            nregs = [
                nc.gpsimd.value_load(
                    nidx_sb[:1, 2 * g + s:2 * g + s + 1],
                    min_val=0, max_val=plan.tc[g][s] * CAP)
                for g in range(NG) for s in range(2)]
            nregs = {2 * g + s: nregs[2 * g + s]
                     for g in range(NG) for s in range(2)}
        gbuf = [None, None]
        if cfg.skip_gather or (cfg.bucket_calls and cfg.trim):
            # memset every physical pool slot once so trimmed tail slots never
            # expose uninitialized SBUF (NaN * 0 = NaN in the PSUM accumulate)
            for s in range(2):
                for _ in range(1 if cfg.skip_gather else cfg.gather_bufs):
                    gbuf[s] = gpool[s].tile([128, GW], bf16, tag=f"gb{s}",
                                            name=f"gbuf{s}")
                    nc.vector.memset(gbuf[s][:], 0.0)

        def fetch_chunk(s, ci):
            ge = cfg.gelem
            # trim the final call to the tiles actually consumed
            n = min(CH, plan.t_stream(s) - ci * CH)
            gbuf[s] = gpool[s].tile([128, GW], bf16, tag=f"gb{s}",
                                    name=f"gbuf{s}")
            nc.gpsimd.dma_gather(
                gbuf[s][:, :n * ge].rearrange("p (k j) -> p k j", j=ge),
                xp[s][:] if ge == D
                else (xp[s].rearrange("a (c d) -> (a c) d", d=ge) if ge < D
                      else xp[s].rearrange("(a c) d -> a (c d)", c=ge // D)),
                gi_sb[s][:, ci * CH * 8:ci * CH * 8 + n * 8],
                n * 128, n * 128, ge,
                single_packet=(CH * 128 <= 1024),
                queue_num=(s * 2 + ci % 2 if cfg.gather_queues == 4
                           else s % cfg.gather_queues),
            )

        def fetch_bucket(s, g, spos0):
            ge = cfg.gelem
            ci = 2 * g + s
            nt = plan.tc[g][s]
            gbuf[s] = gpool[s].tile([128, GW], bf16, tag=f"gb{s}",
                                    name=f"gbuf{s}")
            nreg = nregs[ci] if cfg.trim else nt * 128
            nc.gpsimd.dma_gather(
                gbuf[s][:, :nt * ge].rearrange("p (k j) -> p k j", j=ge),
                xp[s][:],
                gi_sb[s][:, spos0 * 8:(spos0 + nt) * 8],
                nt * 128, nreg, ge,
                single_packet=(nt * 128 <= 1024),
                queue_num=(s * 2 + g % 2 if cfg.gather_queues == 4
                           else s % cfg.gather_queues),
            )

        def body():
            if cfg.skip_compute:
                if cfg.bucket_calls:
                    sp = [0, 0]
                    for g in range(NG):
                        for s in range(2):
                            fetch_bucket(s, g, sp[s])
                            sp[s] += plan.tc[g][s]
                else:
                    for ci in range(max(tp) // CH):
                        for s in range(2):
                            if ci < tp[s] // CH:
                                fetch_chunk(s, ci)
                return
            spos = [0, 0]
            pending = []

            def project(g, agg_sb, rows_g):
                for rc in range(math.ceil(rows_g / 128)):
                    w = min(128, rows_g - rc * 128)
                    out_ps = ps_out.tile([128, D], f32)
                    nc.tensor.matmul(
                        out=out_ps[:w, :],
                        lhsT=agg_sb[:, rc * 128:rc * 128 + w],
                        rhs=wt_ap(),
                        start=True, stop=True,
                    )
                    out_sb = opool.tile([128, D], f32, tag="outsb")
                    nc.vector.tensor_tensor(
                        out=out_sb[:w, :], in0=out_ps[:w, :], in1=bb_ap(w),
                        op=mybir.AluOpType.add,
                    )
                    r0 = g * GR + rc * 128
                    nc.sync.dma_start(out=y[r0:r0 + w, :], in_=out_sb[:w, :])

            for g in range(NG):
                rows_g = min(GR, RPC - g * GR)
                agg = ps_agg.tile([128, GR], f32)
                first = True
                for t in range(max(plan.tc[g])):
                    for s in range(2):
                        if t >= plan.tc[g][s]:
                            continue
                        if not cfg.skip_gather:
                            if cfg.bucket_calls:
                                if t == 0:
                                    fetch_bucket(s, g, spos[s])
                            elif spos[s] % CH == 0:
                                fetch_chunk(s, spos[s] // CH)
                        k = t if cfg.bucket_calls else spos[s] % CH
                        base = plan.db[g][s][t]
                        nc.tensor.matmul(
                            out=agg[:, base:base + BW],
                            lhsT=gbuf[s][:, k * D:(k + 1) * D],
                            rhs=sv_sb[s][:, spos[s] * BW:(spos[s] + 1) * BW],
                            start=first, stop=True,
                            skip_group_check=True,
                        )
                        first = False
                        spos[s] += 1
                if cfg.skip_epilog:
                    continue
                # projection for group g-epi_delay: its PSUM drain finished
                # groups ago, so the PE does not stall on the DVE here
                while pending and pending[0][0] <= g - cfg.epi_delay:
                    project(*pending.pop(0))
                agg_sb = epool.tile([128, GR], pdt_np, tag="aggsb")
                nc.vector.tensor_copy(out=agg_sb[:, :rows_g],
                                      in_=agg[:, :rows_g])
                pending.append((g, agg_sb, rows_g))
            for item in pending:
                project(*item)

        if cfg.loop_n > 0:
            with tc.For_i(0, cfg.loop_n, 1):
                body()
        else:
            for _ in range(cfg.repeats):
                body()

    nc.compile()
    return nc


_CACHE = {}


def _get_nc(cfg: Cfg, plan: Plan):
    key = (cfg, plan)
    if key not in _CACHE:
        _CACHE[key] = _build_nc(cfg, plan)
    return _CACHE[key]


def kernel(x, edge_row, edge_col, edge_val, W, b):
    from concourse.bass_utils import run_bass_kernel_spmd

    cfg = Cfg()
    plan, in_maps = _preprocess(cfg, x, edge_row, edge_col, edge_val, W, b)
    nc = _get_nc(cfg, plan)
    res = run_bass_kernel_spmd(nc, in_maps, core_ids=list(range(cfg.n_cores)))
    RPC = cfg.rows_per_core
    out = np.empty((cfg.n_nodes, D), dtype=np.float32)
    for c in range(cfg.n_cores):
        out[c * RPC:(c + 1) * RPC] = res.results[c]["y"][:RPC]
    return out



# revision 32
# speedup vs baseline: 1.3341x; 1.0755x over previous
"""GCN layer (SpMM + linear) on 8 Trainium2 NeuronCores — exact-packed dest-banded tiles, host-dense selection (bf16).

out[i] = (sum_{e: edge_row[e]==i} edge_val[e] * x[edge_col[e]]) @ W.T + b

Destination rows are partitioned across 8 cores (6250 each) into 13 PSUM
groups of 496 rows.  Per (group, source-half) bucket, edges are sorted by
destination and packed 128 per gather tile — no per-window padding.  Tile t's
destinations fall in a narrow data-derived band [db[t], db[t]+BW); the
selection matrix sval[slot, dest-db] (bf16, host-precomputed, resident in
SBUF) is dense over the band, so duplicate (src,dst) edges just sum.

Each slot gathers one 256B bf16 x row via SWDGE dma_gather (int16 indices,
lo/hi source halves on separate queues), 32 tiles per call to amortize the
~1-2us/call descriptor-generation fixed cost.  matmul(lhsT=gathered, rhs=sval
band) accumulates agg.T[feat, dest] into the group's PSUM bank.  The epilogue
(PSUM drain -> bf16 projection with W.T -> bias add -> DMA out) is
software-pipelined two groups behind the aggregation so the in-order PE queue
never stalls on the DVE's PSUM drain.
"""

import math
from dataclasses import dataclass

import numpy as np

GR = 496          # dest rows per PSUM group
CAP = 128         # slots per gather tile
D = 128           # feature dim


@dataclass(frozen=True)
class Cfg:
    n_nodes: int = 50000
    n_edges: int = 800000
    n_cores: int = 8
    chunk: int = 32           # tiles per dma_gather call
    gather_queues: int = 4
    gather_bufs: int = 6
    agg_bufs: int = 2         # PSUM agg groups in flight
    gelem: int = 128          # gather elems per descriptor (diagnostic only)
    dma_scratch: int = 16384  # SWDGE descriptor ring carveout bytes/partition
    epi_delay: int = 2        # groups the projection trails the aggregation by
    proj_bf16: bool = True    # project in bf16 (agg cast during PSUM drain)
    bucket_calls: bool = False  # one dma_gather per (group, stream) bucket
    trim: bool = False          # runtime-trim gathers to per-core edge counts
    # (bucket_calls+trim cuts ~4% of gather descriptors but the register-length
    #  path wedges the device on this runtime -- left off)
    skip_compute: bool = False
    skip_gather: bool = False   # ablation: matmuls on stale buffers
    skip_epilog: bool = False   # ablation: no PSUM drain / projection / store
    loop_n: int = 0
    repeats: int = 1

    @property
    def split(self) -> int:
        return self.n_nodes // 2

    @property
    def rows_per_core(self) -> int:
        return self.n_nodes // self.n_cores

    @property
    def n_groups(self) -> int:
        return math.ceil(self.rows_per_core / GR)


@dataclass(frozen=True)
class Plan:
    tc: tuple    # tc[g][s]: tiles per (group, stream), max over cores
    db: tuple    # db[g][s][t]: band start column of tile t
    bw: int      # band width (compiled free size of the sval matmul)

    def t_stream(self, s):
        return sum(t[s] for t in self.tc)


def _preprocess(cfg: Cfg, x, edge_row, edge_col, edge_val, W, b):
    import ml_dtypes

    RPC = cfg.rows_per_core
    NG = cfg.n_groups
    SPLIT = cfg.split

    x = np.asarray(x)
    edge_row = np.asarray(edge_row)
    edge_col = np.asarray(edge_col)
    edge_val = np.asarray(edge_val)

    xb = np.ascontiguousarray(x.astype(ml_dtypes.bfloat16))
    xplo = np.ascontiguousarray(xb[:SPLIT])
    xphi = np.ascontiguousarray(xb[SPLIT:])

    # Pass 1: per (core, group, stream) dest-sorted edge arrays.
    per_core = []
    for c in range(cfg.n_cores):
        e0, e1 = np.searchsorted(edge_row, [c * RPC, (c + 1) * RPC])
        r_loc = edge_row[e0:e1] - c * RPC
        g_of_e = r_loc // GR
        d_loc = r_loc - g_of_e * GR
        src = edge_col[e0:e1].astype(np.int64)
        s_of_e = (src >= SPLIT).astype(np.int64)
        val = edge_val[e0:e1].astype(np.float64)
        buckets = {}
        for g in range(NG):
            for s in range(2):
                m = (g_of_e == g) & (s_of_e == s)
                order = np.argsort(d_loc[m], kind="stable")
                buckets[(g, s)] = (src[m][order] - s * SPLIT,
                                  d_loc[m][order], val[m][order])
        per_core.append(buckets)

    # Plan: tile counts, band starts, band width (shared across cores).
    tc = []
    for g in range(NG):
        tc.append(tuple(
            max(math.ceil(len(pc[(g, s)][0]) / CAP) for pc in per_core)
            for s in range(2)))

    db = []
    bw = 0
    for g in range(NG):
        row = []
        for s in range(2):
            nt = tc[g][s]
            starts = np.full(nt, GR, dtype=np.int64)
            ends = np.zeros(nt, dtype=np.int64)
            for pc in per_core:
                dd = pc[(g, s)][1]
                for t in range(nt):
                    seg = dd[t * CAP:(t + 1) * CAP]
                    if len(seg):
                        starts[t] = min(starts[t], seg[0])
                        ends[t] = max(ends[t], seg[-1] + 1)
            starts = np.minimum(starts, ends)  # empty tiles -> band at end
            bw = max(bw, int((ends - starts).max(initial=0)))
            row.append(tuple(int(v) for v in starts))
        db.append(tuple(row))
    bw = math.ceil(bw / 16) * 16
    # clamp band starts so db+bw stays inside the group
    db = tuple(
        tuple(tuple(min(v, GR - bw) for v in row_s) for row_s in row)
        for row in db)
    plan = Plan(tc=tuple(tc), db=db, bw=bw)

    CH = cfg.chunk
    tp = [math.ceil(plan.t_stream(s) / CH) * CH for s in range(2)]

    # projection computed transposed: out.T[of, dest] = wt.T @ agg.T, so the
    # y store writes ~2KB per partition per group instead of 512B per row
    wt = np.asarray(W).T.astype(
        ml_dtypes.bfloat16 if cfg.proj_bf16 else np.float32)
    bb = np.tile(np.asarray(b)[:, None].astype(np.float32), (1, GR))

    def wrap_idx(idx_lin):
        return np.tile(np.ascontiguousarray(idx_lin.reshape(-1, 16).T), (8, 1))

    in_maps = []
    for c in range(cfg.n_cores):
        buckets = per_core[c]
        idx_lin = [np.zeros(tp[s] * CAP, dtype=np.int16) for s in range(2)]
        sval = [np.zeros((128, tp[s] * bw), dtype=np.float64) for s in range(2)]
        spos = [0, 0]
        for g in range(NG):
            for t in range(max(plan.tc[g])):
                for s in range(2):
                    if t >= plan.tc[g][s]:
                        continue
                    ss, dd, vv = buckets[(g, s)]
                    ss = ss[t * CAP:(t + 1) * CAP]
                    dd = dd[t * CAP:(t + 1) * CAP]
                    vv = vv[t * CAP:(t + 1) * CAP]
                    pos = spos[s]
                    base = plan.db[g][s][t]
                    idx_lin[s][pos * CAP:pos * CAP + len(ss)] = ss
                    np.add.at(sval[s],
                              (np.arange(len(ss)), pos * bw + dd - base), vv)
                    spos[s] += 1
        nidx = []
        start = [0, 0]
        for g in range(NG):
            for s in range(2):
                n = len(buckets[(g, s)][0])
                nv = min(math.ceil(n / 16) * 16, plan.tc[g][s] * CAP)
                nidx.append(nv)
                if cfg.trim:
                    # gather contract: num_idxs_reg == count of indices >= 0,
                    # negatives must trail -- mark trimmed pad slots with -1
                    b0 = start[s] * CAP
                    idx_lin[s][b0 + nv:b0 + plan.tc[g][s] * CAP] = -1
                start[s] += plan.tc[g][s]
        gdiv = max(1, cfg.gelem // 128)  # >1: timing-only 512B-desc experiment
        in_maps.append({
            "xplo": xplo, "xphi": xphi,
            "nidx": np.ascontiguousarray(
                np.asarray(nidx, dtype=np.int32)[None, :]),
            "gilo": np.ascontiguousarray(wrap_idx(idx_lin[0] // gdiv)),
            "gihi": np.ascontiguousarray(wrap_idx(idx_lin[1] // gdiv)),
            "svlo": np.ascontiguousarray(
                sval[0].astype(np.float32).astype(
                    np.asarray(xb).dtype)),
            "svhi": np.ascontiguousarray(
                sval[1].astype(np.float32).astype(
                    np.asarray(xb).dtype)),
            "cst": np.ascontiguousarray(bb, dtype=np.float32),
            "wt": np.ascontiguousarray(wt),
        })
    return plan, in_maps


def _build_nc(cfg: Cfg, plan: Plan):
    from contextlib import ExitStack

    import concourse.bacc as bacc
    import concourse.mybir as mybir
    import concourse.tile as tile

    f32 = mybir.dt.float32
    bf16 = mybir.dt.bfloat16
    i16 = mybir.dt.int16
    NG = cfg.n_groups
    RPC = cfg.rows_per_core
    CH = cfg.chunk
    SPLIT = cfg.split
    BW = plan.bw
    tp = [math.ceil(plan.t_stream(s) / CH) * CH for s in range(2)]
    pdt_np = bf16 if cfg.proj_bf16 else f32

    nc = bacc.Bacc("TRN2", target_bir_lowering=False,
                   num_swdge_queues=cfg.gather_queues,
                   dynamic_dma_scratch_size=cfg.dma_scratch)
    xp = [nc.dram_tensor("xplo", [SPLIT, D], bf16, kind="ExternalInput"),
          nc.dram_tensor("xphi", [SPLIT, D], bf16, kind="ExternalInput")]
    gi = [nc.dram_tensor("gilo", [128, tp[0] * 8], i16, kind="ExternalInput"),
          nc.dram_tensor("gihi", [128, tp[1] * 8], i16, kind="ExternalInput")]
    sv = [nc.dram_tensor("svlo", [128, tp[0] * BW], bf16,
                         kind="ExternalInput"),
          nc.dram_tensor("svhi", [128, tp[1] * BW], bf16,
                         kind="ExternalInput")]
    cst = nc.dram_tensor("cst", [128, GR], f32, kind="ExternalInput")
    wtd = nc.dram_tensor("wt", [128, D], pdt_np, kind="ExternalInput")
    i32 = mybir.dt.int32
    NCALLS = 2 * NG
    nidx_d = nc.dram_tensor("nidx", [1, NCALLS], i32, kind="ExternalInput")
    y = nc.dram_tensor("y", [128, NG * GR], f32, kind="ExternalOutput")
    TCMAX = max(max(t) for t in plan.tc)

    with tile.TileContext(nc) as tc, ExitStack() as ctx:
        const = ctx.enter_context(tc.tile_pool(name="const", bufs=1))
        gpool = [ctx.enter_context(tc.tile_pool(name="glo",
                                                bufs=cfg.gather_bufs)),
                 ctx.enter_context(tc.tile_pool(name="ghi",
                                                bufs=cfg.gather_bufs))]
        epool = ctx.enter_context(tc.tile_pool(name="epilog",
                                               bufs=cfg.epi_delay + 1))
        opool = ctx.enter_context(tc.tile_pool(name="outp", bufs=4))
        ps_agg = ctx.enter_context(tc.tile_pool(name="psagg", bufs=cfg.agg_bufs,
                                                space="PSUM"))
        ps_out = ctx.enter_context(tc.tile_pool(name="psout", bufs=4,
                                                space="PSUM"))

        cst_sb = const.tile([128, GR], f32)
        nc.sync.dma_start(out=cst_sb[:], in_=cst[:])
        wt_sb = const.tile([128, D], pdt_np, tag="wt")
        nc.sync.dma_start(out=wt_sb[:], in_=wtd[:])
        nidx_sb = const.tile([1, NCALLS], i32, tag="nidx")
        nc.sync.dma_start(out=nidx_sb[:], in_=nidx_d[:])

        def wt_ap():
            return wt_sb[:, :]

        gi_sb = []
        sv_sb = []
        for s in range(2):
            t = const.tile([128, tp[s] * 8], i16, tag=f"gi{s}")
            nc.sync.dma_start(out=t[:], in_=gi[s][:])
            gi_sb.append(t)
            t = const.tile([128, tp[s] * BW], bf16, tag=f"sv{s}")
            nc.sync.dma_start(out=t[:], in_=sv[s][:])
            sv_sb.append(t)

        GW = (TCMAX if cfg.bucket_calls else CH) * max(D, cfg.gelem)
        nregs = None
        if cfg.bucket_calls and cfg.trim:
            # hoisted: per-core gather lengths, constant across repeats
            nregs = [
                nc.gpsimd.value_load(
                    nidx_sb[:1, 2 * g + s:2 * g + s + 1],
                    min_val=0, max_val=plan.tc[g][s] * CAP)
                for g in range(NG) for s in range(2)]
            nregs = {2 * g + s: nregs[2 * g + s]
                     for g in range(NG) for s in range(2)}
        gbuf = [None, None]
        if cfg.skip_gather or (cfg.bucket_calls and cfg.trim):
            # memset every physical pool slot once so trimmed tail slots never
            # expose uninitialized SBUF (NaN * 0 = NaN in the PSUM accumulate)
            for s in range(2):
                for _ in range(1 if cfg.skip_gather else cfg.gather_bufs):
                    gbuf[s] = gpool[s].tile([128, GW], bf16, tag=f"gb{s}",
                                            name=f"gbuf{s}")
                    nc.vector.memset(gbuf[s][:], 0.0)

        def fetch_chunk(s, ci):
            ge = cfg.gelem
            # trim the final call to the tiles actually consumed
            n = min(CH, plan.t_stream(s) - ci * CH)
            gbuf[s] = gpool[s].tile([128, GW], bf16, tag=f"gb{s}",
                                    name=f"gbuf{s}")
            nc.gpsimd.dma_gather(
                gbuf[s][:, :n * ge].rearrange("p (k j) -> p k j", j=ge),
                xp[s][:] if ge == D
                else (xp[s].rearrange("a (c d) -> (a c) d", d=ge) if ge < D
                      else xp[s].rearrange("(a c) d -> a (c d)", c=ge // D)),
                gi_sb[s][:, ci * CH * 8:ci * CH * 8 + n * 8],
                n * 128, n * 128, ge,
                single_packet=(CH * 128 <= 1024),
                queue_num=(s * 2 + ci % 2 if cfg.gather_queues == 4
                           else s % cfg.gather_queues),
            )

        def fetch_bucket(s, g, spos0):
            ge = cfg.gelem
            ci = 2 * g + s
            nt = plan.tc[g][s]
            gbuf[s] = gpool[s].tile([128, GW], bf16, tag=f"gb{s}",
                                    name=f"gbuf{s}")
            nreg = nregs[ci] if cfg.trim else nt * 128
            nc.gpsimd.dma_gather(
                gbuf[s][:, :nt * ge].rearrange("p (k j) -> p k j", j=ge),
                xp[s][:],
                gi_sb[s][:, spos0 * 8:(spos0 + nt) * 8],
                nt * 128, nreg, ge,
                single_packet=(nt * 128 <= 1024),
                queue_num=(s * 2 + g % 2 if cfg.gather_queues == 4
                           else s % cfg.gather_queues),
            )

        def body():
            if cfg.skip_compute:
                if cfg.bucket_calls:
                    sp = [0, 0]
                    for g in range(NG):
                        for s in range(2):
                            fetch_bucket(s, g, sp[s])
                            sp[s] += plan.tc[g][s]
                else:
                    for ci in range(max(tp) // CH):
                        for s in range(2):
                            if ci < tp[s] // CH:
                                fetch_chunk(s, ci)
                return
            spos = [0, 0]
            pending = []

            def project(g, agg_sb, rows_g):
                # out.T[of, dest] = wt.T @ agg.T -- one matmul per group,
                # stationary wt, and a ~2KB-per-partition store
                out_ps = ps_out.tile([128, GR], f32)
                nc.tensor.matmul(
                    out=out_ps[:, :rows_g],
                    lhsT=wt_ap(),
                    rhs=agg_sb[:, :rows_g],
                    start=True, stop=True,
                )
                out_sb = opool.tile([128, GR], f32, tag="outsb")
                nc.vector.tensor_tensor(
                    out=out_sb[:, :rows_g], in0=out_ps[:, :rows_g],
                    in1=cst_sb[:, :rows_g],
                    op=mybir.AluOpType.add,
                )
                r0 = g * GR
                nc.sync.dma_start(out=y[:, r0:r0 + rows_g],
                                  in_=out_sb[:, :rows_g])

            for g in range(NG):
                rows_g = min(GR, RPC - g * GR)
                agg = ps_agg.tile([128, GR], f32)
                first = True
                for t in range(max(plan.tc[g])):
                    for s in range(2):
                        if t >= plan.tc[g][s]:
                            continue
                        if not cfg.skip_gather:
                            if cfg.bucket_calls:
                                if t == 0:
                                    fetch_bucket(s, g, spos[s])
                            elif spos[s] % CH == 0:
                                fetch_chunk(s, spos[s] // CH)
                        k = t if cfg.bucket_calls else spos[s] % CH
                        base = plan.db[g][s][t]
                        nc.tensor.matmul(
                            out=agg[:, base:base + BW],
                            lhsT=gbuf[s][:, k * D:(k + 1) * D],
                            rhs=sv_sb[s][:, spos[s] * BW:(spos[s] + 1) * BW],
                            start=first, stop=True,
                            skip_group_check=True,
                        )
                        first = False
                        spos[s] += 1
                if cfg.skip_epilog:
                    continue
                # projection for group g-epi_delay: its PSUM drain finished
                # groups ago, so the PE does not stall on the DVE here
                while pending and pending[0][0] <= g - cfg.epi_delay:
                    project(*pending.pop(0))
                agg_sb = epool.tile([128, GR], pdt_np, tag="aggsb")
                nc.vector.tensor_copy(out=agg_sb[:, :rows_g],
                                      in_=agg[:, :rows_g])
                pending.append((g, agg_sb, rows_g))
            for item in pending:
                project(*item)

        if cfg.loop_n > 0:
            with tc.For_i(0, cfg.loop_n, 1):
                body()
        else:
            for _ in range(cfg.repeats):
                body()

    nc.compile()
    return nc


_CACHE = {}


def _get_nc(cfg: Cfg, plan: Plan):
    key = (cfg, plan)
    if key not in _CACHE:
        _CACHE[key] = _build_nc(cfg, plan)
    return _CACHE[key]


def kernel(x, edge_row, edge_col, edge_val, W, b):
    from concourse.bass_utils import run_bass_kernel_spmd

    cfg = Cfg()
    plan, in_maps = _preprocess(cfg, x, edge_row, edge_col, edge_val, W, b)
    nc = _get_nc(cfg, plan)
    res = run_bass_kernel_spmd(nc, in_maps, core_ids=list(range(cfg.n_cores)))
    RPC = cfg.rows_per_core
    out = np.empty((cfg.n_nodes, D), dtype=np.float32)
    for c in range(cfg.n_cores):
        out[c * RPC:(c + 1) * RPC] = res.results[c]["y"][:, :RPC].T
    return out

